# revision 1
# baseline (speedup 1.0000x reference)
"""Trainium2 Bass kernel: single-channel 2D conv (valid), X[8192,8192] * w[5,5] + bias.

Strategy: row-shard X across 8 NeuronCores with a (kh-1)-row halo (host-side
overlapping slices; weight/bias replicated). On each core, the conv is computed
as 5 PSUM-accumulated TensorE matmuls per output tile: for each kernel column
dj, a banded stationary matrix A_dj[k, m] = w[k-m, dj] (0 <= k-m < 5) contracts
over 128 input rows to produce 124 output rows of the column-direction conv,
while the moving operand is the input tile shifted by dj columns. Accumulating
the 5 dj-shifts in PSUM yields the full 5x5 conv. fp32r (hardware rounds
operands to 11 mantissa bits, fp32 accumulate) runs the PE at 1 cycle/row.
"""

import numpy as np

import concourse.bass as bass
import concourse.mybir as mybir
from concourse import bacc
from concourse import bass_utils
from concourse.tile import TileContext

H = 8192
W = 8192
KH = 5
KW = 5
OH = H - KH + 1  # 8188
OW = W - KW + 1  # 8188

NCORES = 8
ROWS_OUT = 1024  # output rows per core (8*1024 = 8192 >= 8188; tail cropped)
ROWS_IN = ROWS_OUT + KH - 1  # 1028

BAND_OUT = 124  # output rows per matmul band (K=128 partitions -> M=124)
SUB_W = 512  # matmul moving free dim (one PSUM bank of fp32)

# 8 full bands of 124 output rows + a 32-row tail band (fp32r handles M=32)
_BANDS = [(124 * i, 124) for i in range(8)] + [(992, 32)]
# 16 uniform column subtiles; the last one overlaps
_SUB_STARTS = [512 * i for i in range(15)] + [OW - SUB_W]

_PROGRAM_CACHE = {}

# Populated by the most recent kernel() call when tracing is enabled via
# TRACE=1 (module attr) — used by test.py for HW exec time reporting.
TRACE = False
LAST_RUN = {}


def _build_program(bias_val: float):
    f32 = mybir.dt.float32
    f32r = mybir.dt.float32r

    nc = bacc.Bacc("TRN2", target_bir_lowering=False, debug=False, num_devices=NCORES)

    Xs = nc.dram_tensor("Xs", [ROWS_IN, W], f32r, kind="ExternalInput")
    Aw = nc.dram_tensor("Aw", [128, KW * BAND_OUT], f32r, kind="ExternalInput")
    # Output rows padded to 8192 cols so every store row is a 32KiB-aligned
    # full-line HBM write; host crops to 8188.
    Y = nc.dram_tensor("Y", [ROWS_OUT, W], f32, kind="ExternalOutput")

    with TileContext(nc) as tc:
        with (
            tc.tile_pool(name="const", bufs=1) as cpool,
            tc.tile_pool(name="inp", bufs=3) as in_pool,
            tc.tile_pool(name="outp", bufs=2) as out_pool,
            tc.tile_pool(name="psum", bufs=8, space="PSUM") as psum_pool,
        ):
            A_t = cpool.tile([128, KW * BAND_OUT], f32r)
            nc.sync.dma_start(A_t[:], Aw.ap())

            # DRAM->SBUF loads spread across all 16 SDMA engines; SBUF->DRAM
            # stores concentrate on few engines per instruction, so issue
            # stores as many small instructions alternating across the two
            # HWDGE rings to engage more engines.
            # Topology: loads on the gpsimd SWDGE queue (32KiB descriptors,
            # spreads over all 16 SDMA engines, never blocked behind
            # compute-dependent stores). Stores mostly on the two HWDGE rings
            # (fast but pinned to SDMA engines 64-71); ~20% of store rows
            # offloaded to SWDGE (deferred one band so they don't block the
            # load issue stream) to relieve the hot engines.
            qs = [nc.sync, nc.scalar]
            pending = []
            for bi, (r0, rows_out) in enumerate(_BANDS):
                rows_in = rows_out + KH - 1
                in_t = in_pool.tile([rows_in, W], f32r)
                nc.gpsimd.dma_start(in_t[:], Xs.ap()[r0 : r0 + rows_in, :])
                if pending:
                    r0s, sw_rows, t = pending.pop(0)
                    nc.gpsimd.dma_start(Y.ap()[r0s : r0s + sw_rows, :], t[0:sw_rows, :])
                out_t = out_pool.tile([rows_out, W], f32)
                for c0 in _SUB_STARTS:
                    ps = psum_pool.tile([rows_out, SUB_W], f32)
                    for dj in range(KW):
                        nc.tensor.matmul(
                            ps[:],
                            A_t[0:rows_in, dj * BAND_OUT : dj * BAND_OUT + rows_out],
                            in_t[:, c0 + dj : c0 + dj + SUB_W],
                            start=(dj == 0),
                            stop=(dj == KW - 1),
                        )
                    dst = out_t[:, c0 : c0 + SUB_W]
                    if bias_val == 0.0:
                        nc.vector.tensor_copy(dst, ps[:])
                    else:
                        nc.scalar.activation(
                            dst,
                            ps[:],
                            mybir.ActivationFunctionType.Copy,
                            bias=bias_val,
                        )
                sw_rows = 24 if rows_out == BAND_OUT else 8
                pending.append((r0, sw_rows, out_t))
                n_chunks = 12 if rows_out == BAND_OUT else 4
                lo0 = sw_rows
                bounds = [
                    lo0 + (rows_out - lo0) * i // n_chunks for i in range(n_chunks + 1)
                ]
                for ci in range(n_chunks):
                    lo, hi = bounds[ci], bounds[ci + 1]
                    qs[ci % 2].dma_start(
                        Y.ap()[r0 + lo : r0 + hi, :], out_t[lo:hi, :]
                    )
            while pending:
                r0s, sw_rows, t = pending.pop(0)
                nc.gpsimd.dma_start(Y.ap()[r0s : r0s + sw_rows, :], t[0:sw_rows, :])

    nc.compile()
    return nc


def kernel(X, weight, bias):
    X = np.ascontiguousarray(np.asarray(X, dtype=np.float32))
    weight = np.asarray(weight, dtype=np.float32)
    bias = np.asarray(bias, dtype=np.float32)
    assert X.shape == (H, W) and weight.shape == (KH, KW)

    bias_val = float(bias.reshape(-1)[0])
    key = bias_val
    nc = _PROGRAM_CACHE.get(key)
    if nc is None:
        nc = _build_program(bias_val)
        _PROGRAM_CACHE[key] = nc

    # Banded stationary matrices: A[k, dj*124 + m] = w[k-m, dj] for 0<=k-m<5
    A = np.zeros((128, KW * BAND_OUT), dtype=np.float32)
    m = np.arange(BAND_OUT)
    for dj in range(KW):
        for di in range(KH):
            A[m + di, dj * BAND_OUT + m] = weight[di, dj]

    # Row-shard with halo; pad the bottom so every core gets ROWS_IN rows.
    Xp = np.zeros((NCORES * ROWS_OUT + KH - 1, W), dtype=np.float32)
    Xp[:H] = X
    in_maps = [
        {"Xs": Xp[c * ROWS_OUT : c * ROWS_OUT + ROWS_IN], "Aw": A}
        for c in range(NCORES)
    ]

    res = bass_utils.run_bass_kernel_spmd(
        nc, in_maps, core_ids=list(range(NCORES)), trace=TRACE
    )
    LAST_RUN.clear()
    LAST_RUN.update(
        exec_time_ns=res.exec_time_ns,
        instructions_and_trace=res.instructions_and_trace,
        profile_json=res.profile_json,
    )

    out = np.concatenate([res.results[c]["Y"] for c in range(NCORES)], axis=0)
    return np.ascontiguousarray(out[:OH, :OW])



# revision 3
# speedup vs baseline: 1.3341x; 1.3341x over previous
"""Trainium2 Bass kernel: single-channel 2D conv (valid), X[8192,8192] * w[5,5] + bias.

Strategy: row-shard X across 8 NeuronCores with a (kh-1)-row halo (host-side
overlapping slices; weight/bias replicated). On each core, the conv is computed
as 5 PSUM-accumulated TensorE matmuls per output tile: for each kernel column
dj, a banded stationary matrix A_dj[k, m] = w[k-m, dj] (0 <= k-m < 5) contracts
over 128 input rows to produce 124 output rows of the column-direction conv,
while the moving operand is the input tile shifted by dj columns. Accumulating
the 5 dj-shifts in PSUM yields the full 5x5 conv.

I/O is bf16 on the wire: the host rounds X (and the banded weights) to bf16,
halving HBM read traffic, and the kernel stores bf16 results that the host
upcasts to fp32. PSUM accumulation stays fp32; total rel-err ~1e-3, well under
the 2e-2 gate. All loads and stores issue as single whole-band SWDGE DMAs so
the partition swizzle spreads each transfer across all 16 SDMA engines
(HWDGE rings only reach engines 0-7). Stores are deferred by one band so the
single SWDGE queue never stalls waiting on compute.
"""

import ml_dtypes
import numpy as np

import concourse.bass as bass
import concourse.mybir as mybir
from concourse import bacc
from concourse import bass_utils
from concourse.tile import TileContext

H = 8192
W = 8192
KH = 5
KW = 5
OH = H - KH + 1  # 8188
OW = W - KW + 1  # 8188

NCORES = 8
ROWS_OUT = 1024  # output rows per core (8*1024 = 8192 >= 8188; tail cropped)
ROWS_IN = ROWS_OUT + KH - 1  # 1028

BAND_OUT = 124  # output rows per matmul band (K=128 partitions -> M=124)
SUB_W = 512  # matmul moving free dim (one PSUM bank of fp32)

# 8 full bands of 124 output rows + a 32-row tail band
_BANDS = [(124 * i, 124) for i in range(8)] + [(992, 32)]
# 16 uniform column subtiles; the last one overlaps
_SUB_STARTS = [512 * i for i in range(15)] + [OW - SUB_W]

_PROGRAM_CACHE = {}

# Populated by the most recent kernel() call when tracing is enabled via
# TRACE=1 (module attr) — used by test.py for HW exec time reporting.
TRACE = False
LAST_RUN = {}

BF16 = ml_dtypes.bfloat16


def _build_program(bias_val: float):
    f32 = mybir.dt.float32
    bf16 = mybir.dt.bfloat16

    nc = bacc.Bacc("TRN2", target_bir_lowering=False, debug=False, num_devices=NCORES)

    Xs = nc.dram_tensor("Xs", [ROWS_IN, W], bf16, kind="ExternalInput")
    Aw = nc.dram_tensor("Aw", [128, KW * BAND_OUT], bf16, kind="ExternalInput")
    # Output rows padded to 8192 cols so every store row is a full contiguous
    # 16KiB HBM write; host crops to 8188.
    Y = nc.dram_tensor("Y", [ROWS_OUT, W], bf16, kind="ExternalOutput")

    with TileContext(nc) as tc:
        with (
            tc.tile_pool(name="const", bufs=1) as cpool,
            tc.tile_pool(name="inp", bufs=3) as in_pool,
            tc.tile_pool(name="outp", bufs=2) as out_pool,
            tc.tile_pool(name="psum", bufs=8, space="PSUM") as psum_pool,
        ):
            A_t = cpool.tile([128, KW * BAND_OUT], bf16)
            nc.sync.dma_start(A_t[:], Aw.ap())

            # One whole-band DMA per load/store: the SDMA partition swizzle
            # spreads a full 124/128-partition transfer evenly across all 16
            # engines. Stores trail the compute by one band so the gpsimd
            # (SWDGE) queue head never blocks on an unfinished PSUM copy.
            pending = []
            for bi, (r0, rows_out) in enumerate(_BANDS):
                rows_in = rows_out + KH - 1
                in_t = in_pool.tile([rows_in, W], bf16)
                nc.gpsimd.dma_start(in_t[:], Xs.ap()[r0 : r0 + rows_in, :])
                if pending:
                    r0s, rows_s, t = pending.pop(0)
                    nc.gpsimd.dma_start(Y.ap()[r0s : r0s + rows_s, :], t[:])
                out_t = out_pool.tile([rows_out, W], bf16)
                for si, c0 in enumerate(_SUB_STARTS):
                    ps = psum_pool.tile([rows_out, SUB_W], f32)
                    for dj in range(KW):
                        nc.tensor.matmul(
                            ps[:],
                            A_t[0:rows_in, dj * BAND_OUT : dj * BAND_OUT + rows_out],
                            in_t[:, c0 + dj : c0 + dj + SUB_W],
                            start=(dj == 0),
                            stop=(dj == KW - 1),
                        )
                    dst = out_t[:, c0 : c0 + SUB_W]
                    if bias_val == 0.0:
                        if si % 2 == 0:
                            nc.vector.tensor_copy(dst, ps[:])
                        else:
                            nc.scalar.activation(
                                dst, ps[:], mybir.ActivationFunctionType.Copy
                            )
                    else:
                        nc.scalar.activation(
                            dst,
                            ps[:],
                            mybir.ActivationFunctionType.Copy,
                            bias=bias_val,
                        )
                pending.append((r0, rows_out, out_t))
            # Drain the deferred store (the tail band): split across the two
            # HWDGE rings to shorten the tail.
            while pending:
                r0s, rows_s, t = pending.pop(0)
                half = rows_s // 2
                nc.sync.dma_start(Y.ap()[r0s : r0s + half, :], t[0:half, :])
                nc.scalar.dma_start(
                    Y.ap()[r0s + half : r0s + rows_s, :], t[half:rows_s, :]
                )

    nc.compile()
    return nc


def kernel(X, weight, bias):
    X = np.ascontiguousarray(np.asarray(X, dtype=np.float32))
    weight = np.asarray(weight, dtype=np.float32)
    bias = np.asarray(bias, dtype=np.float32)
    assert X.shape == (H, W) and weight.shape == (KH, KW)

    bias_val = float(bias.reshape(-1)[0])
    key = bias_val
    nc = _PROGRAM_CACHE.get(key)
    if nc is None:
        nc = _build_program(bias_val)
        _PROGRAM_CACHE[key] = nc

    # Banded stationary matrices: A[k, dj*124 + m] = w[k-m, dj] for 0<=k-m<5
    A = np.zeros((128, KW * BAND_OUT), dtype=np.float32)
    m = np.arange(BAND_OUT)
    for dj in range(KW):
        for di in range(KH):
            A[m + di, dj * BAND_OUT + m] = weight[di, dj]
    A = A.astype(BF16)

    # Row-shard with halo; pad the bottom so every core gets ROWS_IN rows.
    Xp = np.zeros((NCORES * ROWS_OUT + KH - 1, W), dtype=BF16)
    Xp[:H] = X.astype(BF16)
    in_maps = [
        {"Xs": Xp[c * ROWS_OUT : c * ROWS_OUT + ROWS_IN], "Aw": A}
        for c in range(NCORES)
    ]

    res = bass_utils.run_bass_kernel_spmd(
        nc, in_maps, core_ids=list(range(NCORES)), trace=TRACE
    )
    LAST_RUN.clear()
    LAST_RUN.update(
        exec_time_ns=res.exec_time_ns,
        instructions_and_trace=res.instructions_and_trace,
        profile_json=res.profile_json,
    )

    out = np.concatenate([res.results[c]["Y"] for c in range(NCORES)], axis=0)
    return np.ascontiguousarray(out[:OH, :OW].astype(np.float32))


# revision 5
# speedup vs baseline: 1.6188x; 1.2134x over previous
"""Trainium2 Bass kernel: single-channel 2D conv (valid), X[8192,8192] * w[5,5] + bias.

Strategy: row-shard X across 8 NeuronCores with a (kh-1)-row halo (host-side
overlapping slices; weight/bias replicated). On each core, the conv is computed
as 5 PSUM-accumulated TensorE matmuls per output tile: for each kernel column
dj, a banded stationary matrix A_dj[k, m] = w[k-m, dj] (0 <= k-m < 5) contracts
over 128 input rows to produce 124 output rows of the column-direction conv,
while the moving operand is the input tile shifted by dj columns. Accumulating
the 5 dj-shifts in PSUM yields the full 5x5 conv.

I/O is bf16 on the wire: the host rounds X (and the banded weights) to bf16,
halving HBM read traffic, and the kernel stores bf16 results that the host
upcasts to fp32. PSUM accumulation stays fp32; total rel-err ~3e-3, well under
the 2e-2 gate. All bulk loads and stores issue as whole-band SWDGE DMAs so the
partition swizzle spreads each transfer across all 16 SDMA engines (HWDGE
rings only reach engines 0-7). Loads are split column-wise so the first
matmuls only wait on a quarter-tile; stores are deferred by one band so the
single SWDGE queue never stalls waiting on compute. The 32 leftover rows
(1024 = 8*124 + 32) are produced by a 9th full-height band shifted up to rows
900..1028, which avoids the slow K=36 partial-array matmuls; only its last 32
rows are copied out and stored.
"""

import ml_dtypes
import numpy as np

import concourse.bass as bass
import concourse.mybir as mybir
from concourse import bacc
from concourse import bass_utils
from concourse.tile import TileContext

H = 8192
W = 8192
KH = 5
KW = 5
OH = H - KH + 1  # 8188
OW = W - KW + 1  # 8188

NCORES = 8
ROWS_OUT = 1024  # output rows per core (8*1024 = 8192 >= 8188; tail cropped)
ROWS_IN = ROWS_OUT + KH - 1  # 1028

BAND_OUT = 124  # output rows per matmul band (K=128 partitions -> M=124)
BAND_IN = BAND_OUT + KH - 1  # 128
SUB_W = 512  # matmul moving free dim (one PSUM bank of fp32)

# 8 full bands + a 9th full band shifted up so it ends exactly at row 1024.
# (in_row0, copy_lo): copy/store rows [copy_lo, 124) of the band's output.
_BANDS = [(124 * i, 0) for i in range(8)] + [(ROWS_OUT - BAND_OUT, 92)]
# 16 uniform column subtiles; the last one overlaps
_SUB_STARTS = [512 * i for i in range(15)] + [OW - SUB_W]

_PROGRAM_CACHE = {}

# Populated by the most recent kernel() call when tracing is enabled via
# TRACE=1 (module attr) — used by test.py for HW exec time reporting.
TRACE = False
LAST_RUN = {}

BF16 = ml_dtypes.bfloat16


def _load_splits(bi):
    # Column split points for the band's input load. Subtile s reads columns
    # [512s, 512s + 516), so split points sit at 512s + 4 to keep every
    # subtile's dependency to the minimal set of load pieces.
    if bi == 0:
        return [0, 2052, 4100, 6148, W]
    return [0, 4100, W]


def _build_program(bias_val: float):
    f32 = mybir.dt.float32
    bf16 = mybir.dt.bfloat16

    nc = bacc.Bacc("TRN2", target_bir_lowering=False, debug=False, num_devices=NCORES)

    Xs = nc.dram_tensor("Xs", [ROWS_IN, W], bf16, kind="ExternalInput")
    Aw = nc.dram_tensor("Aw", [128, KW * BAND_OUT], bf16, kind="ExternalInput")
    # Output rows padded to 8192 cols so every store row is a full contiguous
    # 16KiB HBM write; host crops to 8188.
    Y = nc.dram_tensor("Y", [ROWS_OUT, W], bf16, kind="ExternalOutput")

    with TileContext(nc) as tc:
        with (
            tc.tile_pool(name="const", bufs=1) as cpool,
            tc.tile_pool(name="inp", bufs=3) as in_pool,
            tc.tile_pool(name="outp", bufs=3) as out_pool,
            tc.tile_pool(name="psum", bufs=8, space="PSUM") as psum_pool,
        ):
            A_t = cpool.tile([128, KW * BAND_OUT], bf16)
            nc.sync.dma_start(A_t[:], Aw.ap())

            pending = []
            for bi, (r0, copy_lo) in enumerate(_BANDS):
                in_t = in_pool.tile([BAND_IN, W], bf16)
                splits = _load_splits(bi)
                for c0, c1 in zip(splits, splits[1:]):
                    nc.gpsimd.dma_start(
                        in_t[:, c0:c1], Xs.ap()[r0 : r0 + BAND_IN, c0:c1]
                    )
                if pending:
                    y0, lo, t = pending.pop(0)
                    nc.gpsimd.dma_start(
                        Y.ap()[y0 + lo : y0 + BAND_OUT, :], t[lo:BAND_OUT, :]
                    )
                out_t = out_pool.tile([BAND_OUT, W], bf16)
                # Engine (DVE/ACT) partition bases must be 32-aligned; the
                # store slice below may start at an unaligned row, so the
                # PSUM evacuation rounds down to a 32-aligned row.
                cl = (copy_lo // 32) * 32
                for si, c0 in enumerate(_SUB_STARTS):
                    ps = psum_pool.tile([BAND_OUT, SUB_W], f32)
                    for dj in range(KW):
                        nc.tensor.matmul(
                            ps[:],
                            A_t[0:BAND_IN, dj * BAND_OUT : (dj + 1) * BAND_OUT],
                            in_t[:, c0 + dj : c0 + dj + SUB_W],
                            start=(dj == 0),
                            stop=(dj == KW - 1),
                        )
                    dst = out_t[cl:BAND_OUT, c0 : c0 + SUB_W]
                    src = ps[cl:BAND_OUT, :]
                    if bias_val == 0.0:
                        if si % 2 == 0:
                            nc.vector.tensor_copy(dst, src)
                        else:
                            nc.scalar.activation(
                                dst, src, mybir.ActivationFunctionType.Copy
                            )
                    else:
                        nc.scalar.activation(
                            dst,
                            src,
                            mybir.ActivationFunctionType.Copy,
                            bias=bias_val,
                        )
                pending.append((r0, copy_lo, out_t))
            # Drain the deferred store (the shifted tail band): only rows
            # [92, 124) are fresh; split across HWDGE+SWDGE so the two halves
            # land on disjoint SDMA engine sets in parallel.
            while pending:
                y0, lo, t = pending.pop(0)
                mid = (lo + BAND_OUT) // 2
                nc.sync.dma_start(Y.ap()[y0 + lo : y0 + mid, :], t[lo:mid, :])
                nc.gpsimd.dma_start(
                    Y.ap()[y0 + mid : y0 + BAND_OUT, :], t[mid:BAND_OUT, :]
                )

    nc.compile()
    return nc


def kernel(X, weight, bias):
    X = np.ascontiguousarray(np.asarray(X, dtype=np.float32))
    weight = np.asarray(weight, dtype=np.float32)
    bias = np.asarray(bias, dtype=np.float32)
    assert X.shape == (H, W) and weight.shape == (KH, KW)

    bias_val = float(bias.reshape(-1)[0])
    key = bias_val
    nc = _PROGRAM_CACHE.get(key)
    if nc is None:
        nc = _build_program(bias_val)
        _PROGRAM_CACHE[key] = nc

    # Banded stationary matrices: A[k, dj*124 + m] = w[k-m, dj] for 0<=k-m<5
    A = np.zeros((128, KW * BAND_OUT), dtype=np.float32)
    m = np.arange(BAND_OUT)
    for dj in range(KW):
        for di in range(KH):
            A[m + di, dj * BAND_OUT + m] = weight[di, dj]
    A = A.astype(BF16)

    # Row-shard with halo; pad the bottom so every core gets ROWS_IN rows.
    Xp = np.zeros((NCORES * ROWS_OUT + KH - 1, W), dtype=BF16)
    Xp[:H] = X.astype(BF16)
    in_maps = [
        {"Xs": Xp[c * ROWS_OUT : c * ROWS_OUT + ROWS_IN], "Aw": A}
        for c in range(NCORES)
    ]

    res = bass_utils.run_bass_kernel_spmd(
        nc, in_maps, core_ids=list(range(NCORES)), trace=TRACE
    )
    LAST_RUN.clear()
    LAST_RUN.update(
        exec_time_ns=res.exec_time_ns,
        instructions_and_trace=res.instructions_and_trace,
        profile_json=res.profile_json,
    )

    out = np.concatenate([res.results[c]["Y"] for c in range(NCORES)], axis=0)
    return np.ascontiguousarray(out[:OH, :OW].astype(np.float32))


# revision 9
# speedup vs baseline: 1.6354x; 1.0103x over previous
"""Trainium2 Bass kernel: single-channel 2D conv (valid), X[8192,8192] * w[5,5] + bias.

Strategy: row-shard X across 8 NeuronCores with a (kh-1)-row halo (host-side
overlapping slices; weight/bias replicated). On each core, the conv is computed
as 5 PSUM-accumulated TensorE matmuls per output tile: for each kernel column
dj, a banded stationary matrix A_dj[k, m] = w[k-m, dj] (0 <= k-m < 5) contracts
over 128 input rows to produce 124 output rows of the column-direction conv,
while the moving operand is the input tile shifted by dj columns. Accumulating
the 5 dj-shifts in PSUM yields the full 5x5 conv.

I/O is bf16 on the wire: the host rounds X (and the banded weights) to bf16,
halving HBM read traffic, and the kernel stores bf16 results that the host
upcasts to fp32. PSUM accumulation stays fp32; total rel-err ~3e-3, well under
the 2e-2 gate. All bulk loads and stores issue as whole-band SWDGE DMAs so the
partition swizzle spreads each transfer across all 16 SDMA engines (HWDGE
rings only reach engines 0-7). Loads are split column-wise so the first
matmuls only wait on a quarter-tile; stores are deferred by one band so the
single SWDGE queue never stalls waiting on compute. The 32 leftover rows
(1024 = 8*124 + 32) are produced by a 9th full-height band shifted up to rows
900..1028, which avoids the slow K=36 partial-array matmuls; only its last 32
rows are copied out and stored.
"""

import ml_dtypes
import numpy as np

import concourse.bass as bass
import concourse.mybir as mybir
from concourse import bacc
from concourse import bass_utils
from concourse.tile import TileContext

H = 8192
W = 8192
KH = 5
KW = 5
OH = H - KH + 1  # 8188
OW = W - KW + 1  # 8188

NCORES = 8
ROWS_OUT = 1024  # output rows per core (8*1024 = 8192 >= 8188; tail cropped)
ROWS_IN = ROWS_OUT + KH - 1  # 1028

BAND_OUT = 124  # output rows per matmul band (K=128 partitions -> M=124)
BAND_IN = BAND_OUT + KH - 1  # 128
SUB_W = 512  # matmul moving free dim (one PSUM bank of fp32)

# 8 full bands + a 9th full band shifted up so it ends exactly at row 1024.
# (in_row0, copy_lo): copy/store rows [copy_lo, 124) of the band's output.
_BANDS = [(124 * i, 0) for i in range(8)] + [(ROWS_OUT - BAND_OUT, 92)]
# 16 uniform column subtiles; the last one overlaps
_SUB_STARTS = [512 * i for i in range(15)] + [OW - SUB_W]

_PROGRAM_CACHE = {}

# Populated by the most recent kernel() call when tracing is enabled via
# TRACE=1 (module attr) — used by test.py for HW exec time reporting.
TRACE = False
LAST_RUN = {}

BF16 = ml_dtypes.bfloat16


def _load_splits(bi):
    # Column split points for the band's input load. Subtile s reads columns
    # [512s, 512s + 516), so split points sit at 512s + 4 to keep every
    # subtile's dependency to the minimal set of load pieces.
    if bi == 0:
        return [0, 2052, 4100, 6148, W]
    return [0, 4100, W]


def _build_program(bias_val: float):
    f32 = mybir.dt.float32
    bf16 = mybir.dt.bfloat16

    nc = bacc.Bacc("TRN2", target_bir_lowering=False, debug=False, num_devices=NCORES)

    Xs = nc.dram_tensor("Xs", [ROWS_IN, W], bf16, kind="ExternalInput")
    Aw = nc.dram_tensor("Aw", [128, KW * BAND_OUT], bf16, kind="ExternalInput")
    # Output rows padded to 8192 cols so every store row is a full contiguous
    # 16KiB HBM write; host crops to 8188.
    Y = nc.dram_tensor("Y", [ROWS_OUT, W], bf16, kind="ExternalOutput")

    with TileContext(nc) as tc:
        with (
            tc.tile_pool(name="const", bufs=1) as cpool,
            tc.tile_pool(name="inp", bufs=3) as in_pool,
            tc.tile_pool(name="outp", bufs=3) as out_pool,
            tc.tile_pool(name="psum", bufs=8, space="PSUM") as psum_pool,
        ):
            A_t = cpool.tile([128, KW * BAND_OUT], bf16)
            nc.sync.dma_start(A_t[:], Aw.ap())

            # PE warm-up: ~4us of junk matmuls flips the HAM clock gate to
            # 8/8 (2.4 GHz) before the first input tile lands, so band 0 runs
            # at the warm rate. Results land in rotating PSUM banks and are
            # overwritten by the first real accumulation groups.
            for _ in range(10):
                wps = psum_pool.tile([BAND_OUT, SUB_W], f32, name="ps")
                nc.tensor.matmul(
                    wps[:], A_t[0:128, 0:BAND_OUT], A_t[0:128, 0:SUB_W],
                    start=True, stop=True,
                )

            pending = []
            for bi, (r0, copy_lo) in enumerate(_BANDS):
                in_t = in_pool.tile([BAND_IN, W], bf16)
                splits = _load_splits(bi)
                for c0, c1 in zip(splits, splits[1:]):
                    nc.gpsimd.dma_start(
                        in_t[:, c0:c1], Xs.ap()[r0 : r0 + BAND_IN, c0:c1]
                    )
                if pending:
                    y0, lo, t = pending.pop(0)
                    nc.gpsimd.dma_start(
                        Y.ap()[y0 + lo : y0 + BAND_OUT, :], t[lo:BAND_OUT, :]
                    )
                out_t = out_pool.tile([BAND_OUT, W], bf16)
                # Engine (DVE/ACT) partition bases must be 32-aligned; the
                # store slice below may start at an unaligned row, so the
                # PSUM evacuation rounds down to a 32-aligned row.
                cl = (copy_lo // 32) * 32
                for si, c0 in enumerate(_SUB_STARTS):
                    ps = psum_pool.tile([BAND_OUT, SUB_W], f32)
                    for dj in range(KW):
                        nc.tensor.matmul(
                            ps[:],
                            A_t[0:BAND_IN, dj * BAND_OUT : (dj + 1) * BAND_OUT],
                            in_t[:, c0 + dj : c0 + dj + SUB_W],
                            start=(dj == 0),
                            stop=(dj == KW - 1),
                        )
                    dst = out_t[cl:BAND_OUT, c0 : c0 + SUB_W]
                    src = ps[cl:BAND_OUT, :]
                    if bias_val == 0.0:
                        if si % 2 == 0:
                            nc.vector.tensor_copy(dst, src)
                        else:
                            nc.scalar.activation(
                                dst, src, mybir.ActivationFunctionType.Copy
                            )
                    else:
                        nc.scalar.activation(
                            dst,
                            src,
                            mybir.ActivationFunctionType.Copy,
                            bias=bias_val,
                        )
                pending.append((r0, copy_lo, out_t))
            # Drain the deferred store (the shifted tail band): only rows
            # [92, 124) are fresh. HWDGE rings only — the gpsimd end-of-kernel
            # DRAIN costs ~11us after its queue's last transfer completes, so
            # the final store must not ride SWDGE.
            while pending:
                y0, lo, t = pending.pop(0)
                mid = (lo + BAND_OUT) // 2
                nc.sync.dma_start(Y.ap()[y0 + lo : y0 + mid, :], t[lo:mid, :])
                nc.scalar.dma_start(
                    Y.ap()[y0 + mid : y0 + BAND_OUT, :], t[mid:BAND_OUT, :]
                )

    nc.compile()
    return nc


def kernel(X, weight, bias):
    X = np.ascontiguousarray(np.asarray(X, dtype=np.float32))
    weight = np.asarray(weight, dtype=np.float32)
    bias = np.asarray(bias, dtype=np.float32)
    assert X.shape == (H, W) and weight.shape == (KH, KW)

    bias_val = float(bias.reshape(-1)[0])
    key = bias_val
    nc = _PROGRAM_CACHE.get(key)
    if nc is None:
        nc = _build_program(bias_val)
        _PROGRAM_CACHE[key] = nc

    # Banded stationary matrices: A[k, dj*124 + m] = w[k-m, dj] for 0<=k-m<5
    A = np.zeros((128, KW * BAND_OUT), dtype=np.float32)
    m = np.arange(BAND_OUT)
    for dj in range(KW):
        for di in range(KH):
            A[m + di, dj * BAND_OUT + m] = weight[di, dj]
    A = A.astype(BF16)

    # Row-shard with halo; pad the bottom so every core gets ROWS_IN rows.
    Xp = np.zeros((NCORES * ROWS_OUT + KH - 1, W), dtype=BF16)
    Xp[:H] = X.astype(BF16)
    in_maps = [
        {"Xs": Xp[c * ROWS_OUT : c * ROWS_OUT + ROWS_IN], "Aw": A}
        for c in range(NCORES)
    ]

    res = bass_utils.run_bass_kernel_spmd(
        nc, in_maps, core_ids=list(range(NCORES)), trace=TRACE
    )
    LAST_RUN.clear()
    LAST_RUN.update(
        exec_time_ns=res.exec_time_ns,
        instructions_and_trace=res.instructions_and_trace,
        profile_json=res.profile_json,
    )

    out = np.concatenate([res.results[c]["Y"] for c in range(NCORES)], axis=0)
    return np.ascontiguousarray(out[:OH, :OW].astype(np.float32))


# revision 10
# speedup vs baseline: 1.6413x; 1.0036x over previous
"""Trainium2 Bass kernel: single-channel 2D conv (valid), X[8192,8192] * w[5,5] + bias.

v4: row-shard the first 7936 output rows across 8 NeuronCores (8 full
124-row matmul bands per core, 992 rows each, with a 4-row halo); the
remaining 252-row bottom strip is sharded column-wise, each core taking a
[256 x 1032] patch. This keeps every TensorE matmul a full-size
K=128/M=124/N=512 stream (the leftover rows cost 45 small-strip matmuls
per core instead of an 80-matmul 9th band).

Per output tile, the conv runs as 5 PSUM-accumulated TensorE matmuls: for
each kernel column dj, a banded stationary A_dj[k, m] = w[k-m, dj]
(0 <= k-m < 5) contracts over 128 input rows to produce 124 output rows of
the column-direction conv, with the moving operand the input tile shifted
by dj columns.

I/O is bf16 on the wire (host casts; rel-err ~3e-3 vs the 2e-2 gate).
Bulk loads and mid-kernel stores ride single whole-band SWDGE DMAs (the
partition swizzle spreads them across all 16 SDMA engines); the final
stores ride the two HWDGE rings because the gpsimd end-of-kernel DRAIN
costs ~11us after its queue's last transfer completes. A burst of warm-up
matmuls flips the PE HAM clock gate to 2.4 GHz before the first tile lands.
"""

import ml_dtypes
import numpy as np

import concourse.bass as bass
import concourse.mybir as mybir
from concourse import bacc
from concourse import bass_utils
from concourse.tile import TileContext

H = 8192
W = 8192
KH = 5
KW = 5
OH = H - KH + 1  # 8188
OW = W - KW + 1  # 8188

NCORES = 8
BAND_OUT = 124  # output rows per matmul band (K=128 partitions -> M=124)
BAND_IN = BAND_OUT + KH - 1  # 128
NBANDS = 8
ROWS_MAIN = NBANDS * BAND_OUT  # 992 output rows per core
MAIN_IN = ROWS_MAIN + KH - 1  # 996 input rows per core
SUB_W = 512  # matmul moving free dim (one PSUM bank of fp32)

# Bottom strip: output rows [7936, 8188), each core takes 1024 output cols.
STRIP_R0 = NCORES * ROWS_MAIN  # 7936
STRIP_ROWS = OH - STRIP_R0  # 252
STRIP_IN_ROWS = STRIP_ROWS + KH - 1  # 256
STRIP_W = 1024  # output cols per core
STRIP_IN_W = STRIP_W + 4 + 4  # 1032: +4 conv halo, +4 subtile-2 dj reach
# Strip bands: (band_row0, store_lo) — band 2 overlaps band 1, storing only
# its last 4 rows.
_STRIP_BANDS = [(0, 0), (124, 0), (STRIP_ROWS - BAND_OUT, 120)]
_STRIP_SUBS = [0, 512, STRIP_W + 4 - SUB_W]  # 0, 512, 516

_SUB_STARTS = [512 * i for i in range(15)] + [OW - SUB_W]

_PROGRAM_CACHE = {}

TRACE = False
LAST_RUN = {}

BF16 = ml_dtypes.bfloat16


def _load_splits(bi):
    # Column split points for a main band's input load; subtile s reads
    # columns [512s, 512s + 516), so splits at 512s + 4 keep each subtile's
    # dependency to the minimal set of pieces.
    if bi == 0:
        return [0, 2052, 4100, 6148, W]
    return [0, 4100, W]


def _build_program(bias_val: float):
    f32 = mybir.dt.float32
    bf16 = mybir.dt.bfloat16

    nc = bacc.Bacc("TRN2", target_bir_lowering=False, debug=False, num_devices=NCORES)

    Xs = nc.dram_tensor("Xs", [MAIN_IN, W], bf16, kind="ExternalInput")
    Xt = nc.dram_tensor("Xt", [STRIP_IN_ROWS, STRIP_IN_W], bf16, kind="ExternalInput")
    Aw = nc.dram_tensor("Aw", [128, KW * BAND_OUT], bf16, kind="ExternalInput")
    # Row-padded outputs; host crops (8192 -> 8188 cols, 1028 -> 1024 cols).
    Y = nc.dram_tensor("Y", [ROWS_MAIN, W], bf16, kind="ExternalOutput")
    Yt = nc.dram_tensor("Yt", [STRIP_ROWS, STRIP_W + 4], bf16, kind="ExternalOutput")

    def copy_out(dst, src, si):
        if bias_val == 0.0:
            if si % 2 == 0:
                nc.vector.tensor_copy(dst, src)
            else:
                nc.scalar.activation(dst, src, mybir.ActivationFunctionType.Copy)
        else:
            nc.scalar.activation(
                dst, src, mybir.ActivationFunctionType.Copy, bias=bias_val
            )

    with TileContext(nc) as tc:
        with (
            tc.tile_pool(name="const", bufs=1) as cpool,
            tc.tile_pool(name="inp", bufs=3) as in_pool,
            tc.tile_pool(name="strip_inp", bufs=3) as sin_pool,
            tc.tile_pool(name="outp", bufs=3) as out_pool,
            tc.tile_pool(name="strip_outp", bufs=3) as sout_pool,
            tc.tile_pool(name="psum", bufs=8, space="PSUM") as psum_pool,
        ):
            A_t = cpool.tile([128, KW * BAND_OUT], bf16)
            nc.sync.dma_start(A_t[:], Aw.ap())

            # PE warm-up: ~4us of junk matmuls flips the HAM clock gate to
            # 8/8 (2.4 GHz) before the first input tile lands. Results land
            # in rotating PSUM banks and are overwritten by the first real
            # accumulation groups (start=True clears the bank).
            for _ in range(10):
                wps = psum_pool.tile([BAND_OUT, SUB_W], f32, name="ps")
                nc.tensor.matmul(
                    wps[:], A_t[0:128, 0:BAND_OUT], A_t[0:128, 0:SUB_W],
                    start=True, stop=True,
                )

            strip_tiles = []
            pending = []
            for bi in range(NBANDS):
                r0 = BAND_OUT * bi
                in_t = in_pool.tile([BAND_IN, W], bf16)
                splits = _load_splits(bi)
                for c0, c1 in zip(splits, splits[1:]):
                    nc.gpsimd.dma_start(
                        in_t[:, c0:c1], Xs.ap()[r0 : r0 + BAND_IN, c0:c1]
                    )
                if bi == 1:
                    # The strip input tiles are tiny; stage them right after
                    # band 1's load so the strip phase never waits.
                    for sr0, _ in _STRIP_BANDS:
                        st = sin_pool.tile([BAND_IN, STRIP_IN_W], bf16)
                        nc.gpsimd.dma_start(
                            st[:], Xt.ap()[sr0 : sr0 + BAND_IN, :]
                        )
                        strip_tiles.append(st)
                if pending:
                    y0, t = pending.pop(0)
                    nc.gpsimd.dma_start(Y.ap()[y0 : y0 + BAND_OUT, :], t[:])
                out_t = out_pool.tile([BAND_OUT, W], bf16)
                for si, c0 in enumerate(_SUB_STARTS):
                    ps = psum_pool.tile([BAND_OUT, SUB_W], f32)
                    for dj in range(KW):
                        nc.tensor.matmul(
                            ps[:],
                            A_t[0:BAND_IN, dj * BAND_OUT : (dj + 1) * BAND_OUT],
                            in_t[:, c0 + dj : c0 + dj + SUB_W],
                            start=(dj == 0),
                            stop=(dj == KW - 1),
                        )
                    copy_out(out_t[:, c0 : c0 + SUB_W], ps[:], si)
                pending.append((r0, out_t))

            # Bottom strip: 3 bands x 3 subtiles on this core's column patch.
            # Band 7's store rides SWDGE here (the last SWDGE op, finishing
            # well before the strip does); strip stores ride HWDGE.
            y0, t = pending.pop(0)
            nc.gpsimd.dma_start(Y.ap()[y0 : y0 + BAND_OUT, :], t[:])
            for sbi, ((sr0, store_lo), st) in enumerate(
                zip(_STRIP_BANDS, strip_tiles)
            ):
                cl = (store_lo // 32) * 32
                so_t = sout_pool.tile([BAND_OUT, STRIP_W + 4], bf16)
                for si, c0 in enumerate(_STRIP_SUBS):
                    ps = psum_pool.tile([BAND_OUT, SUB_W], f32, name="ps")
                    for dj in range(KW):
                        nc.tensor.matmul(
                            ps[:],
                            A_t[0:BAND_IN, dj * BAND_OUT : (dj + 1) * BAND_OUT],
                            st[:, c0 + dj : c0 + dj + SUB_W],
                            start=(dj == 0),
                            stop=(dj == KW - 1),
                        )
                    copy_out(so_t[cl:BAND_OUT, c0 : c0 + SUB_W], ps[cl:BAND_OUT, :], si)
                nrows = BAND_OUT - store_lo
                hi = store_lo + (nrows + 1) // 2
                q0, q1 = (nc.sync, nc.scalar) if sbi % 2 == 0 else (nc.scalar, nc.sync)
                q0.dma_start(
                    Yt.ap()[sr0 + store_lo : sr0 + hi, :], so_t[store_lo:hi, :]
                )
                if hi < BAND_OUT:
                    q1.dma_start(
                        Yt.ap()[sr0 + hi : sr0 + BAND_OUT, :], so_t[hi:BAND_OUT, :]
                    )

    nc.compile()
    return nc


def kernel(X, weight, bias):
    X = np.ascontiguousarray(np.asarray(X, dtype=np.float32))
    weight = np.asarray(weight, dtype=np.float32)
    bias = np.asarray(bias, dtype=np.float32)
    assert X.shape == (H, W) and weight.shape == (KH, KW)

    bias_val = float(bias.reshape(-1)[0])
    nc = _PROGRAM_CACHE.get(bias_val)
    if nc is None:
        nc = _build_program(bias_val)
        _PROGRAM_CACHE[bias_val] = nc

    # Banded stationary matrices: A[k, dj*124 + m] = w[k-m, dj] for 0<=k-m<5
    A = np.zeros((128, KW * BAND_OUT), dtype=np.float32)
    m = np.arange(BAND_OUT)
    for dj in range(KW):
        for di in range(KH):
            A[m + di, dj * BAND_OUT + m] = weight[di, dj]
    A = A.astype(BF16)

    Xb = X.astype(BF16)
    strip = np.zeros((NCORES, STRIP_IN_ROWS, STRIP_IN_W), dtype=BF16)
    for c in range(NCORES):
        c0 = c * STRIP_W
        c1 = min(W, c0 + STRIP_IN_W)
        strip[c, :, : c1 - c0] = Xb[STRIP_R0:H, c0:c1]
    in_maps = [
        {"Xs": Xb[c * ROWS_MAIN : c * ROWS_MAIN + MAIN_IN], "Xt": strip[c], "Aw": A}
        for c in range(NCORES)
    ]

    res = bass_utils.run_bass_kernel_spmd(
        nc, in_maps, core_ids=list(range(NCORES)), trace=TRACE
    )
    LAST_RUN.clear()
    LAST_RUN.update(
        exec_time_ns=res.exec_time_ns,
        instructions_and_trace=res.instructions_and_trace,
        profile_json=res.profile_json,
    )

    out = np.empty((OH, OW), dtype=np.float32)
    main = np.concatenate([res.results[c]["Y"] for c in range(NCORES)], axis=0)
    out[:STRIP_R0] = main[:, :OW].astype(np.float32)
    stripe = np.concatenate(
        [res.results[c]["Yt"][:, :STRIP_W] for c in range(NCORES)], axis=1
    )
    out[STRIP_R0:] = stripe[:, :OW].astype(np.float32)
    return out


# revision 11
# speedup vs baseline: 1.6874x; 1.0281x over previous
"""Trainium2 Bass kernel: single-channel 2D conv (valid), X[8192,8192] * w[5,5] + bias.

v4: row-shard the first 7936 output rows across 8 NeuronCores (8 full
124-row matmul bands per core, 992 rows each, with a 4-row halo); the
remaining 252-row bottom strip is sharded column-wise, each core taking a
[256 x 1032] patch. This keeps every TensorE matmul a full-size
K=128/M=124/N=512 stream (the leftover rows cost 45 small-strip matmuls
per core instead of an 80-matmul 9th band).

Per output tile, the conv runs as 5 PSUM-accumulated TensorE matmuls: for
each kernel column dj, a banded stationary A_dj[k, m] = w[k-m, dj]
(0 <= k-m < 5) contracts over 128 input rows to produce 124 output rows of
the column-direction conv, with the moving operand the input tile shifted
by dj columns.

I/O is bf16 on the wire (host casts; rel-err ~3e-3 vs the 2e-2 gate).
Bulk loads and mid-kernel stores ride single whole-band SWDGE DMAs (the
partition swizzle spreads them across all 16 SDMA engines); the final
stores ride the two HWDGE rings because the gpsimd end-of-kernel DRAIN
costs ~11us after its queue's last transfer completes. A burst of warm-up
matmuls flips the PE HAM clock gate to 2.4 GHz before the first tile lands.
"""

import ml_dtypes
import numpy as np

import concourse.bass as bass
import concourse.mybir as mybir
from concourse import bacc
from concourse import bass_utils
from concourse.tile import TileContext

H = 8192
W = 8192
KH = 5
KW = 5
OH = H - KH + 1  # 8188
OW = W - KW + 1  # 8188

NCORES = 8
BAND_OUT = 124  # output rows per matmul band (K=128 partitions -> M=124)
BAND_IN = BAND_OUT + KH - 1  # 128
NBANDS = 8
ROWS_MAIN = NBANDS * BAND_OUT  # 992 output rows per core
MAIN_IN = ROWS_MAIN + KH - 1  # 996 input rows per core
SUB_W = 512  # matmul moving free dim (one PSUM bank of fp32)

# Bottom strip: output rows [7936, 8188), each core takes 1024 output cols.
STRIP_R0 = NCORES * ROWS_MAIN  # 7936
STRIP_ROWS = OH - STRIP_R0  # 252
STRIP_IN_ROWS = STRIP_ROWS + KH - 1  # 256
STRIP_W = 1024  # output cols per core
STRIP_IN_W = STRIP_W + 4 + 4  # 1032: +4 conv halo, +4 subtile-2 dj reach
# Strip bands: (band_row0, store_lo) — band 2 overlaps band 1, storing only
# its last 4 rows.
_STRIP_BANDS = [(0, 0), (124, 0), (STRIP_ROWS - BAND_OUT, 120)]
_STRIP_SUBS = [0, 512, STRIP_W + 4 - SUB_W]  # 0, 512, 516

_SUB_STARTS = [512 * i for i in range(15)] + [OW - SUB_W]

_PROGRAM_CACHE = {}

TRACE = False
LAST_RUN = {}

BF16 = ml_dtypes.bfloat16


def _load_splits(bi):
    # Column split points for a main band's input load; subtile s reads
    # columns [512s, 512s + 516), so splits at 512s + 4 keep each subtile's
    # dependency to the minimal set of pieces.
    if bi == 0:
        return [0, 2052, 4100, 6148, W]
    return [0, 4100, W]


def _build_program(bias_val: float):
    f32 = mybir.dt.float32
    bf16 = mybir.dt.bfloat16

    nc = bacc.Bacc("TRN2", target_bir_lowering=False, debug=False, num_devices=NCORES)

    Xs = nc.dram_tensor("Xs", [MAIN_IN, W], bf16, kind="ExternalInput")
    Xt = nc.dram_tensor("Xt", [STRIP_IN_ROWS, STRIP_IN_W], bf16, kind="ExternalInput")
    Aw = nc.dram_tensor("Aw", [128, KW * BAND_OUT], bf16, kind="ExternalInput")
    # Row-padded outputs; host crops (8192 -> 8188 cols, 1028 -> 1024 cols).
    Y = nc.dram_tensor("Y", [ROWS_MAIN, W], bf16, kind="ExternalOutput")
    Yt = nc.dram_tensor("Yt", [STRIP_ROWS, STRIP_W + 4], bf16, kind="ExternalOutput")

    def copy_out(dst, src, si):
        if bias_val == 0.0:
            if si % 2 == 0:
                nc.vector.tensor_copy(dst, src)
            else:
                nc.scalar.activation(dst, src, mybir.ActivationFunctionType.Copy)
        else:
            nc.scalar.activation(
                dst, src, mybir.ActivationFunctionType.Copy, bias=bias_val
            )

    with TileContext(nc) as tc:
        with (
            tc.tile_pool(name="const", bufs=1) as cpool,
            tc.tile_pool(name="inp", bufs=3) as in_pool,
            tc.tile_pool(name="strip_inp", bufs=3) as sin_pool,
            tc.tile_pool(name="outp", bufs=3) as out_pool,
            tc.tile_pool(name="strip_outp", bufs=3) as sout_pool,
            tc.tile_pool(name="psum", bufs=8, space="PSUM") as psum_pool,
        ):
            A_t = cpool.tile([128, KW * BAND_OUT], bf16)
            nc.sync.dma_start(A_t[:], Aw.ap())

            # PE warm-up: ~4us of junk matmuls flips the HAM clock gate to
            # 8/8 (2.4 GHz) before the first input tile lands. Results land
            # in rotating PSUM banks and are overwritten by the first real
            # accumulation groups (start=True clears the bank).
            for _ in range(10):
                wps = psum_pool.tile([BAND_OUT, SUB_W], f32, name="ps")
                nc.tensor.matmul(
                    wps[:], A_t[0:128, 0:BAND_OUT], A_t[0:128, 0:SUB_W],
                    start=True, stop=True,
                )

            strip_tiles = []
            pending = []
            for bi in range(NBANDS):
                r0 = BAND_OUT * bi
                in_t = in_pool.tile([BAND_IN, W], bf16)
                splits = _load_splits(bi)
                for c0, c1 in zip(splits, splits[1:]):
                    nc.gpsimd.dma_start(
                        in_t[:, c0:c1], Xs.ap()[r0 : r0 + BAND_IN, c0:c1]
                    )
                if bi == 1:
                    # The strip input tiles are tiny; stage them right after
                    # band 1's load so the strip phase never waits.
                    for sr0, _ in _STRIP_BANDS:
                        st = sin_pool.tile([BAND_IN, STRIP_IN_W], bf16)
                        nc.gpsimd.dma_start(
                            st[:], Xt.ap()[sr0 : sr0 + BAND_IN, :]
                        )
                        strip_tiles.append(st)
                if pending:
                    y0, t = pending.pop(0)
                    nc.gpsimd.dma_start(Y.ap()[y0 : y0 + BAND_OUT, :], t[:])
                out_t = out_pool.tile([BAND_OUT, W], bf16)
                for si, c0 in enumerate(_SUB_STARTS):
                    ps = psum_pool.tile([BAND_OUT, SUB_W], f32)
                    for dj in range(KW):
                        nc.tensor.matmul(
                            ps[:],
                            A_t[0:BAND_IN, dj * BAND_OUT : (dj + 1) * BAND_OUT],
                            in_t[:, c0 + dj : c0 + dj + SUB_W],
                            start=(dj == 0),
                            stop=(dj == KW - 1),
                        )
                    copy_out(out_t[:, c0 : c0 + SUB_W], ps[:], si)
                if bi == NBANDS - 1:
                    # Store the last band eagerly in column chunks (chunk g
                    # only depends on subtiles 4g..4g+3's copies) so its
                    # data is off-chip by the time the strip finishes —
                    # outstanding stores at kernel end skew the final
                    # cross-core barrier.
                    for g in range(4):
                        ca, cb = 2048 * g, 2048 * (g + 1)
                        nc.gpsimd.dma_start(
                            Y.ap()[r0 : r0 + BAND_OUT, ca:cb], out_t[:, ca:cb]
                        )
                else:
                    pending.append((r0, out_t))

            # Bottom strip: 3 bands x 3 subtiles on this core's column patch.
            # Strip stores ride HWDGE (the gpsimd DRAIN costs ~11us after the
            # last SWDGE transfer completes).
            for sbi, ((sr0, store_lo), st) in enumerate(
                zip(_STRIP_BANDS, strip_tiles)
            ):
                cl = (store_lo // 32) * 32
                so_t = sout_pool.tile([BAND_OUT, STRIP_W + 4], bf16)
                for si, c0 in enumerate(_STRIP_SUBS):
                    ps = psum_pool.tile([BAND_OUT, SUB_W], f32, name="ps")
                    for dj in range(KW):
                        nc.tensor.matmul(
                            ps[:],
                            A_t[0:BAND_IN, dj * BAND_OUT : (dj + 1) * BAND_OUT],
                            st[:, c0 + dj : c0 + dj + SUB_W],
                            start=(dj == 0),
                            stop=(dj == KW - 1),
                        )
                    copy_out(so_t[cl:BAND_OUT, c0 : c0 + SUB_W], ps[cl:BAND_OUT, :], si)
                nrows = BAND_OUT - store_lo
                hi = store_lo + (nrows + 1) // 2
                q0, q1 = (nc.sync, nc.scalar) if sbi % 2 == 0 else (nc.scalar, nc.sync)
                q0.dma_start(
                    Yt.ap()[sr0 + store_lo : sr0 + hi, :], so_t[store_lo:hi, :]
                )
                if hi < BAND_OUT:
                    q1.dma_start(
                        Yt.ap()[sr0 + hi : sr0 + BAND_OUT, :], so_t[hi:BAND_OUT, :]
                    )

    nc.compile()
    return nc


def kernel(X, weight, bias):
    X = np.ascontiguousarray(np.asarray(X, dtype=np.float32))
    weight = np.asarray(weight, dtype=np.float32)
    bias = np.asarray(bias, dtype=np.float32)
    assert X.shape == (H, W) and weight.shape == (KH, KW)

    bias_val = float(bias.reshape(-1)[0])
    nc = _PROGRAM_CACHE.get(bias_val)
    if nc is None:
        nc = _build_program(bias_val)
        _PROGRAM_CACHE[bias_val] = nc

    # Banded stationary matrices: A[k, dj*124 + m] = w[k-m, dj] for 0<=k-m<5
    A = np.zeros((128, KW * BAND_OUT), dtype=np.float32)
    m = np.arange(BAND_OUT)
    for dj in range(KW):
        for di in range(KH):
            A[m + di, dj * BAND_OUT + m] = weight[di, dj]
    A = A.astype(BF16)

    Xb = X.astype(BF16)
    strip = np.zeros((NCORES, STRIP_IN_ROWS, STRIP_IN_W), dtype=BF16)
    for c in range(NCORES):
        c0 = c * STRIP_W
        c1 = min(W, c0 + STRIP_IN_W)
        strip[c, :, : c1 - c0] = Xb[STRIP_R0:H, c0:c1]
    in_maps = [
        {"Xs": Xb[c * ROWS_MAIN : c * ROWS_MAIN + MAIN_IN], "Xt": strip[c], "Aw": A}
        for c in range(NCORES)
    ]

    res = bass_utils.run_bass_kernel_spmd(
        nc, in_maps, core_ids=list(range(NCORES)), trace=TRACE
    )
    LAST_RUN.clear()
    LAST_RUN.update(
        exec_time_ns=res.exec_time_ns,
        instructions_and_trace=res.instructions_and_trace,
        profile_json=res.profile_json,
    )

    out = np.empty((OH, OW), dtype=np.float32)
    main = np.concatenate([res.results[c]["Y"] for c in range(NCORES)], axis=0)
    out[:STRIP_R0] = main[:, :OW].astype(np.float32)
    stripe = np.concatenate(
        [res.results[c]["Yt"][:, :STRIP_W] for c in range(NCORES)], axis=1
    )
    out[STRIP_R0:] = stripe[:, :OW].astype(np.float32)
    return out


# revision 15
# speedup vs baseline: 1.7715x; 1.0498x over previous
"""Trainium2 Bass kernel: single-channel 2D conv (valid), X[8192,8192] * w[5,5] + bias.

v4: row-shard the first 7936 output rows across 8 NeuronCores (8 full
124-row matmul bands per core, 992 rows each, with a 4-row halo); the
remaining 252-row bottom strip is sharded column-wise, each core taking a
[256 x 1032] patch. This keeps every TensorE matmul a full-size
K=128/M=124/N=512 stream (the leftover rows cost 45 small-strip matmuls
per core instead of an 80-matmul 9th band).

Per output tile, the conv runs as 5 PSUM-accumulated TensorE matmuls: for
each kernel column dj, a banded stationary A_dj[k, m] = w[k-m, dj]
(0 <= k-m < 5) contracts over 128 input rows to produce 124 output rows of
the column-direction conv, with the moving operand the input tile shifted
by dj columns.

I/O is bf16 on the wire (host casts; rel-err ~3e-3 vs the 2e-2 gate).
Bulk loads and mid-kernel stores ride single whole-band SWDGE DMAs (the
partition swizzle spreads them across all 16 SDMA engines); the final
stores ride the two HWDGE rings because the gpsimd end-of-kernel DRAIN
costs ~11us after its queue's last transfer completes. A burst of warm-up
matmuls flips the PE HAM clock gate to 2.4 GHz before the first tile lands.
"""

import ml_dtypes
import numpy as np

import concourse.bass as bass
import concourse.mybir as mybir
from concourse import bacc
from concourse import bass_utils
from concourse.tile import TileContext

H = 8192
W = 8192
KH = 5
KW = 5
OH = H - KH + 1  # 8188
OW = W - KW + 1  # 8188

NCORES = 8
BAND_OUT = 124  # output rows per matmul band (K=128 partitions -> M=124)
BAND_IN = BAND_OUT + KH - 1  # 128
NBANDS = 8
ROWS_MAIN = NBANDS * BAND_OUT  # 992 output rows per core
MAIN_IN = ROWS_MAIN + KH - 1  # 996 input rows per core
SUB_W = 512  # matmul moving free dim (one PSUM bank of fp32)

# Bottom strip: output rows [7936, 8188), each core takes 1024 output cols.
STRIP_R0 = NCORES * ROWS_MAIN  # 7936
STRIP_ROWS = OH - STRIP_R0  # 252
STRIP_IN_ROWS = STRIP_ROWS + KH - 1  # 256
STRIP_W = 1024  # output cols per core
STRIP_IN_W = STRIP_W + 4 + 4  # 1032: +4 conv halo, +4 subtile-2 dj reach
# Strip bands: (band_row0, store_lo) — band 2 overlaps band 1, storing only
# its last 4 rows.
_STRIP_BANDS = [(0, 0), (124, 0), (STRIP_ROWS - BAND_OUT, 120)]
_STRIP_SUBS = [0, 512, STRIP_W + 4 - SUB_W]  # 0, 512, 516

_SUB_STARTS = [512 * i for i in range(15)] + [OW - SUB_W]

_PROGRAM_CACHE = {}

TRACE = False
LAST_RUN = {}

BF16 = ml_dtypes.bfloat16


def _load_splits(bi):
    # Column split points for a main band's input load; subtile s reads
    # columns [512s, 512s + 516), so splits at 512s + 4 keep each subtile's
    # dependency to the minimal set of pieces.
    if bi == 0:
        return [0, 516, 1028, 2052, 4100, W]
    return [0, 4100, W]


def _build_program(bias_val: float):
    f32 = mybir.dt.float32
    bf16 = mybir.dt.bfloat16

    nc = bacc.Bacc("TRN2", target_bir_lowering=False, debug=False, num_devices=NCORES)

    Xs = nc.dram_tensor("Xs", [MAIN_IN, W], bf16, kind="ExternalInput")
    Xt = nc.dram_tensor("Xt", [STRIP_IN_ROWS, STRIP_IN_W], bf16, kind="ExternalInput")
    Aw = nc.dram_tensor("Aw", [128, KW * BAND_OUT], bf16, kind="ExternalInput")
    # Row-padded outputs; host crops (8192 -> 8188 cols, 1028 -> 1024 cols).
    Y = nc.dram_tensor("Y", [ROWS_MAIN, W], bf16, kind="ExternalOutput")
    Yt = nc.dram_tensor("Yt", [STRIP_ROWS, STRIP_W + 4], bf16, kind="ExternalOutput")

    def copy_out(dst, src, si):
        if bias_val == 0.0:
            if si % 2 == 0:
                nc.vector.tensor_copy(dst, src)
            else:
                nc.scalar.activation(dst, src, mybir.ActivationFunctionType.Copy)
        else:
            nc.scalar.activation(
                dst, src, mybir.ActivationFunctionType.Copy, bias=bias_val
            )

    with TileContext(nc) as tc:
        with (
            tc.tile_pool(name="const", bufs=1) as cpool,
            tc.tile_pool(name="inp", bufs=3) as in_pool,
            tc.tile_pool(name="strip_inp", bufs=3) as sin_pool,
            tc.tile_pool(name="outp", bufs=3) as out_pool,
            tc.tile_pool(name="strip_outp", bufs=3) as sout_pool,
            tc.tile_pool(name="psum", bufs=8, space="PSUM") as psum_pool,
        ):
            A_t = cpool.tile([128, KW * BAND_OUT], bf16)
            nc.sync.dma_start(A_t[:], Aw.ap())

            # PE warm-up: ~4us of junk matmuls flips the HAM clock gate to
            # 8/8 (2.4 GHz) before the first input tile lands. The operands
            # are an uninitialized scratch tile (no dependencies, so the
            # warm-up starts immediately); results land in rotating PSUM
            # banks and are overwritten by the first real accumulation
            # groups (start=True clears the bank).
            for _ in range(10):
                wps = psum_pool.tile([BAND_OUT, SUB_W], f32, name="ps")
                nc.tensor.matmul(
                    wps[:], A_t[0:128, 0:BAND_OUT], A_t[0:128, 0:SUB_W],
                    start=True, stop=True,
                )

            def load_band(bi):
                r0 = BAND_OUT * bi
                in_t = in_pool.tile([BAND_IN, W], bf16, name="in_t")
                splits = _load_splits(bi)
                for c0, c1 in zip(splits, splits[1:]):
                    nc.gpsimd.dma_start(
                        in_t[:, c0:c1], Xs.ap()[r0 : r0 + BAND_IN, c0:c1]
                    )
                return in_t

            # Loads run two bands ahead of the store chunks in the SWDGE
            # FIFO so a store's copy-wait never delays a load's issue.
            in_tiles = {0: load_band(0), 1: load_band(1)}
            strip_tiles = []
            for sr0, _ in _STRIP_BANDS:
                st = sin_pool.tile([BAND_IN, STRIP_IN_W], bf16, name="st")
                nc.gpsimd.dma_start(st[:], Xt.ap()[sr0 : sr0 + BAND_IN, :])
                strip_tiles.append(st)

            for bi in range(NBANDS):
                r0 = BAND_OUT * bi
                if bi + 2 < NBANDS:
                    in_tiles[bi + 2] = load_band(bi + 2)
                in_t = in_tiles.pop(bi)
                out_t = out_pool.tile([BAND_OUT, W], bf16)
                for si, c0 in enumerate(_SUB_STARTS):
                    ps = psum_pool.tile([BAND_OUT, SUB_W], f32)
                    for dj in range(KW):
                        nc.tensor.matmul(
                            ps[:],
                            A_t[0:BAND_IN, dj * BAND_OUT : (dj + 1) * BAND_OUT],
                            in_t[:, c0 + dj : c0 + dj + SUB_W],
                            start=(dj == 0),
                            stop=(dj == KW - 1),
                        )
                    copy_out(out_t[:, c0 : c0 + SUB_W], ps[:], si)
                # Store eagerly in two column chunks: chunk g only depends on
                # subtiles 8g..8g+7's copies, so the store pipeline stays fed
                # throughout the band instead of bunching 2MB at kernel end
                # (outstanding stores at the end skew the final cross-core
                # barrier).
                for g in range(2):
                    ca, cb = 4096 * g, 4096 * (g + 1)
                    nc.gpsimd.dma_start(
                        Y.ap()[r0 : r0 + BAND_OUT, ca:cb], out_t[:, ca:cb]
                    )

            # Bottom strip: 3 bands x 3 subtiles on this core's column patch.
            # Strip stores ride HWDGE (the gpsimd DRAIN costs ~11us after the
            # last SWDGE transfer completes).
            for sbi, ((sr0, store_lo), st) in enumerate(
                zip(_STRIP_BANDS, strip_tiles)
            ):
                cl = (store_lo // 32) * 32
                so_t = sout_pool.tile([BAND_OUT, STRIP_W + 4], bf16)
                for si, c0 in enumerate(_STRIP_SUBS):
                    ps = psum_pool.tile([BAND_OUT, SUB_W], f32, name="ps")
                    for dj in range(KW):
                        nc.tensor.matmul(
                            ps[:],
                            A_t[0:BAND_IN, dj * BAND_OUT : (dj + 1) * BAND_OUT],
                            st[:, c0 + dj : c0 + dj + SUB_W],
                            start=(dj == 0),
                            stop=(dj == KW - 1),
                        )
                    copy_out(so_t[cl:BAND_OUT, c0 : c0 + SUB_W], ps[cl:BAND_OUT, :], si)
                q = nc.sync if sbi % 2 == 0 else nc.scalar
                q.dma_start(
                    Yt.ap()[sr0 + store_lo : sr0 + BAND_OUT, :],
                    so_t[store_lo:BAND_OUT, :],
                )

    nc.compile()
    return nc


def kernel(X, weight, bias):
    X = np.ascontiguousarray(np.asarray(X, dtype=np.float32))
    weight = np.asarray(weight, dtype=np.float32)
    bias = np.asarray(bias, dtype=np.float32)
    assert X.shape == (H, W) and weight.shape == (KH, KW)

    bias_val = float(bias.reshape(-1)[0])
    nc = _PROGRAM_CACHE.get(bias_val)
    if nc is None:
        nc = _build_program(bias_val)
        _PROGRAM_CACHE[bias_val] = nc

    # Banded stationary matrices: A[k, dj*124 + m] = w[k-m, dj] for 0<=k-m<5
    A = np.zeros((128, KW * BAND_OUT), dtype=np.float32)
    m = np.arange(BAND_OUT)
    for dj in range(KW):
        for di in range(KH):
            A[m + di, dj * BAND_OUT + m] = weight[di, dj]
    A = A.astype(BF16)

    Xb = X.astype(BF16)
    strip = np.zeros((NCORES, STRIP_IN_ROWS, STRIP_IN_W), dtype=BF16)
    for c in range(NCORES):
        c0 = c * STRIP_W
        c1 = min(W, c0 + STRIP_IN_W)
        strip[c, :, : c1 - c0] = Xb[STRIP_R0:H, c0:c1]
    in_maps = [
        {"Xs": Xb[c * ROWS_MAIN : c * ROWS_MAIN + MAIN_IN], "Xt": strip[c], "Aw": A}
        for c in range(NCORES)
    ]

    res = bass_utils.run_bass_kernel_spmd(
        nc, in_maps, core_ids=list(range(NCORES)), trace=TRACE
    )
    LAST_RUN.clear()
    LAST_RUN.update(
        exec_time_ns=res.exec_time_ns,
        instructions_and_trace=res.instructions_and_trace,
        profile_json=res.profile_json,
    )

    out = np.empty((OH, OW), dtype=np.float32)
    main = np.concatenate([res.results[c]["Y"] for c in range(NCORES)], axis=0)
    out[:STRIP_R0] = main[:, :OW].astype(np.float32)
    stripe = np.concatenate(
        [res.results[c]["Yt"][:, :STRIP_W] for c in range(NCORES)], axis=1
    )
    out[STRIP_R0:] = stripe[:, :OW].astype(np.float32)
    return out
